# revision 10
# baseline (speedup 1.0000x reference)
"""Trainium2 Bass kernel for nn_Attention_16028817948779.

Reference computation (b=4, c=256, heads=8, d=64, h=w=48, n=2304):
  qkv = w_qkv @ x          (1x1 conv)
  q,k,v -> [b, H, d, n];  q,k l2-normalized along n (spatial)
  sim  = (q^T k) * 10;  attn = softmax(sim, axis=-1)
  out  = attn @ v^T -> [b, H, n, d] -> [b, H*d, h, w]
  y    = w_out @ out + b_out

Sharding: 8 cores; core c handles batch c//2, head group (c%2)*4..+4.
Each core computes a partial y over its 4 heads; host sums the two
partials per batch and adds the bias.

Kernel design:
  - All big matmuls in float32r (1 PE cycle/row when free dim >= 256).
  - Attention in transposed form ST[j,i] = k_j . q_i, so softmax needs no
    on-chip transposes: exp without max-subtraction is safe because q,k
    are l2-normalized along n (|sim| <~ 0.7) and the softmax denominator
    comes for free from a ones-column appended to V^T in the PV matmul.
  - Two heads' ST matmuls run concurrently on the PE via row tiling
    (K=64 each; the head pair lives at partitions 0-63 / 64-127 of the
    QKV projection output).
  - 10/(|q_row| |k_row|) folded into a single per-row scale of q.
"""

import os
import sys

import numpy as np

_TRN_REPO = "/opt/trn_rl_repo"
if _TRN_REPO not in sys.path:
    sys.path.insert(0, _TRN_REPO)

B = 4
C = 256
HEADS = 8
D = 64
N = 2304  # 48*48
HID = HEADS * D  # 512

N_CORES = 8
CI = 2  # c chunks of 128
# i/n chunks of <=512 (PSUM bank / fp32 moving-operand limit)
NCHUNKS = [(0, 512), (512, 512), (1024, 512), (1536, 512), (2048, 256)]
NJ = N // 128  # 18 key chunks of 128


def _apply_compat_patches():
    """walrus in this env only accepts ~1 sync wait per instruction, but the
    Tile framework attaches one wait per outstanding proc to a single
    instruction. Split excess waits onto EventSemaphore instructions at the
    BIR-JSON level (Bass.to_json_bytes is the serialization choke point for
    both the native and the axon/PJRT compile paths)."""
    import json

    import concourse.bass as bass

    if getattr(bass.Bass.to_json_bytes, "_waitsplit", False):
        return

    MAXW = 1
    _orig = bass.Bass.to_json_bytes

    def _split_waits(raw):
        m = json.loads(raw)
        ctr = 0
        changed = False
        for f in m.get("functions", []):
            for blk in f.get("blocks", []):
                new_insts = []
                for ins in blk.get("instructions", []):
                    si = ins.get("sync_info")
                    waits = (si or {}).get("on_wait") or []
                    if len(waits) > MAXW:
                        changed = True
                        for w in waits[:-MAXW]:
                            ctr += 1
                            new_insts.append(
                                {
                                    "debug": ins.get("debug", 0),
                                    "engine": ins["engine"],
                                    "ins": [],
                                    "outs": [],
                                    "name": f"waitsplit_{ctr}",
                                    "opcode": "EventSemaphore",
                                    "sync_info": {"on_update": [], "on_wait": [w]},
                                }
                            )
                        si["on_wait"] = waits[-MAXW:]
                    new_insts.append(ins)
                blk["instructions"] = new_insts
        return json.dumps(m).encode() if changed else raw

    def _patched(self):
        return _split_waits(_orig(self))

    _patched._waitsplit = True
    bass.Bass.to_json_bytes = _patched


def build_kernel():
    import concourse.bass as bass
    import concourse.mybir as mybir
    import concourse.tile as tile

    _apply_compat_patches()

    f32 = mybir.dt.float32
    f32r = mybir.dt.float32r
    Exp = mybir.ActivationFunctionType.Exp
    Sqrt = mybir.ActivationFunctionType.Sqrt
    Square = mybir.ActivationFunctionType.Square
    mult = mybir.AluOpType.mult
    X = mybir.AxisListType.X

    nc = bass.Bass()
    x_d = nc.dram_tensor("x", [C, N], f32r, kind="ExternalInput")
    wqT_d = nc.dram_tensor("wqT", [C, 256], f32r, kind="ExternalInput")
    wkT_d = nc.dram_tensor("wkT", [C, 256], f32r, kind="ExternalInput")
    wvT_d = nc.dram_tensor("wvT", [C, 256], f32r, kind="ExternalInput")
    woutT_d = nc.dram_tensor("woutT", [64, 4, 256], f32r, kind="ExternalInput")
    y_d = nc.dram_tensor("y", [C, N], f32, kind="ExternalOutput")

    with tile.TileContext(nc) as tc:
        with (
            tc.tile_pool(name="persist", bufs=1) as pp,
            tc.tile_pool(name="pt", bufs=3) as ptp,
            tc.tile_pool(name="misc", bufs=2) as mp,
            tc.tile_pool(name="dram", bufs=2, space="DRAM") as dp,
            tc.tile_pool(name="ps_st", bufs=2, space="PSUM") as ps_st,
            tc.tile_pool(name="ps_pv", bufs=2, space="PSUM") as ps_pv,
        ):
            # ---- load inputs ----
            x_sb = pp.tile([128, CI, N], f32r)
            for ci in range(CI):
                nc.sync.dma_start(
                    out=x_sb[:, ci, :], in_=x_d[ci * 128 : (ci + 1) * 128, :]
                )
            wq_sb = pp.tile([128, CI, 256], f32r)
            wk_sb = pp.tile([128, CI, 256], f32r)
            wv_sb = pp.tile([128, CI, 256], f32r)
            for w_sb, w_d in ((wq_sb, wqT_d), (wk_sb, wkT_d), (wv_sb, wvT_d)):
                nc.sync.dma_start(
                    out=w_sb[:], in_=w_d.rearrange("(ci p) o -> p ci o", p=128)
                )
            wo_sb = pp.tile([64, 4, 256], f32r)
            nc.sync.dma_start(out=wo_sb[:], in_=woutT_d[:])

            ones_f = pp.tile([128, 1], f32)
            nc.vector.memset(ones_f[:], 1.0)

            # ---- QKV projection ----
            # q_sb/k_sb: [d-part, head-pair, n]; heads 2p at part 0-63, 2p+1 at 64-127
            q_sb = pp.tile([128, 2, N], f32r)
            k_sb = pp.tile([128, 2, N], f32r)
            # per-(tensor, oc) partial sum-of-squares, one column per n-chunk,
            # computed chunk-wise right behind the projection so the PE never
            # waits on the l2norm reduction later
            ssq = mp.tile([128, 2, 2, len(NCHUNKS)], f32, tag="ssq")
            scratch = pp.tile([128, 512], f32)
            for ti, (dst, w_sb) in enumerate(((q_sb, wq_sb), (k_sb, wk_sb))):
                for oc in range(2):
                    for nci, (ns, nl) in enumerate(NCHUNKS):
                        ps = ps_pv.tile([128, 512], f32, tag="pvA", name="qk_ps")
                        for ci in range(CI):
                            nc.tensor.matmul(
                                ps[:, :nl],
                                lhsT=w_sb[:, ci, oc * 128 : (oc + 1) * 128],
                                rhs=x_sb[:, ci, ns : ns + nl],
                                start=(ci == 0),
                                stop=(ci == CI - 1),
                            )
                        nc.vector.tensor_copy(dst[:, oc, ns : ns + nl], ps[:, :nl])
                        nc.scalar.activation(
                            scratch[:, :nl],
                            ps[:, :nl],
                            Square,
                            accum_out=ssq[:, ti, oc, nci : nci + 1],
                        )

            # ---- fold l2norm + SCALE into q: q *= 10/sqrt(ssq_q*ssq_k) per row ----
            sqk = mp.tile([128, 2, 2], f32, tag="sqk")
            nc.vector.reduce_sum(
                sqk.rearrange("p a b -> p (a b)"),
                ssq.rearrange("p a b c -> p (a b) c"),
                axis=X,
            )
            qscale = mp.tile([128, 2], f32, tag="qscale")
            nc.vector.tensor_tensor(qscale[:], sqk[:, 0, :], sqk[:, 1, :], mult)
            # sqrt(0.01*sq*sk) = 0.1*sqrt(..) ; recip -> 10/sqrt(..)
            nc.scalar.activation(qscale[:], qscale[:], Sqrt, scale=0.01)
            nc.vector.reciprocal(qscale[:], qscale[:])
            with nc.allow_low_precision(reason="q scale written as fp32r"):
                for oc in range(2):
                    nc.vector.tensor_scalar_mul(
                        q_sb[:, oc, :], q_sb[:, oc, :], qscale[:, oc : oc + 1]
                    )

            # vt_sb: [n-part, j-chunk, 4*65]; per head 64 v columns + a ones col
            vt_sb = pp.tile([128, NJ, 260], f32r)
            vt4 = vt_sb.rearrange("p j (h e) -> p j h e", e=65)
            nc.vector.tensor_copy(
                vt4[:, :, :, 64:65],
                ones_f[:, 0:1].unsqueeze(1).unsqueeze(1).to_broadcast((128, NJ, 4, 1)),
            )
            for j in range(NJ):
                ps = ps_pv.tile([128, 256], f32, tag="pvB", name="vt_ps")
                for ci in range(CI):
                    nc.tensor.matmul(
                        ps[:],
                        lhsT=x_sb[:, ci, j * 128 : (j + 1) * 128],
                        rhs=wv_sb[:, ci, :],
                        start=(ci == 0),
                        stop=(ci == CI - 1),
                    )
                nc.vector.tensor_copy(
                    vt4[:, j, :, 0:64], ps.rearrange("p (h d) -> p h d", h=4)
                )

            # ---- attention per head pair p (local heads 2p, 2p+1) ----
            outT = [
                pp.tile([64, N], f32r, name=f"outT{h}", tag=f"outT{h}")
                for h in range(4)
            ]
            for ns, il in NCHUNKS:
                for p in range(2):
                    hA, hB = 2 * p, 2 * p + 1
                    pvA = ps_pv.tile([65, 512], f32, tag="pvA", name="pvA")
                    pvB = ps_pv.tile([65, 512], f32, tag="pvB", name="pvB")
                    for j in range(NJ):
                        st = ps_st.tile([128, 2, 512], f32, tag="st", name="st")
                        nc.tensor.matmul(
                            st[:, 0, :il],
                            lhsT=k_sb[0:64, p, j * 128 : (j + 1) * 128],
                            rhs=q_sb[0:64, p, ns : ns + il],
                        )
                        nc.tensor.matmul(
                            st[:, 1, :il],
                            lhsT=k_sb[64:128, p, j * 128 : (j + 1) * 128],
                            rhs=q_sb[64:128, p, ns : ns + il],
                        )
                        pt = ptp.tile([128, 2, 512], f32r, tag="pt", name="pt")
                        nc.scalar.activation(pt[:, :, :il], st[:, :, :il], Exp)
                        nc.tensor.matmul(
                            pvA[:, :il],
                            lhsT=vt4[:, j, hA, :],
                            rhs=pt[:, 0, :il],
                            start=(j == 0),
                            stop=(j == NJ - 1),
                        )
                        nc.tensor.matmul(
                            pvB[:, :il],
                            lhsT=vt4[:, j, hB, :],
                            rhs=pt[:, 1, :il],
                            start=(j == 0),
                            stop=(j == NJ - 1),
                        )
                    # normalize: rows 0-63 divided by row 64 (softmax denom);
                    # the [1,il] reciprocal row is partition-replicated via DMA
                    for pv, h in ((pvA, hA), (pvB, hB)):
                        rden_f = mp.tile([1, 512], f32, tag="rdenf", name="rden_f")
                        nc.vector.reciprocal(rden_f[:, :il], pv[64:65, :il])
                        rden_d = dp.tile([1, 512], f32, tag="rdend", name="rden_d")
                        nc.sync.dma_start(out=rden_d[:, :il], in_=rden_f[:, :il])
                        bc_sb = mp.tile([64, 512], f32, tag="bcsb", name="bc_sb")
                        nc.sync.dma_start(
                            out=bc_sb[:, :il],
                            in_=rden_d[0:1, :il].to_broadcast((64, il)),
                        )
                        nc.vector.tensor_tensor(
                            outT[h][:, ns : ns + il],
                            pv[0:64, :il],
                            bc_sb[:, :il],
                            mult,
                        )

                # ---- output projection for this i-block ----
                for oc in range(2):
                    yps = ps_pv.tile([128, 512], f32, tag="pvA", name="yps")
                    for h in range(4):
                        nc.tensor.matmul(
                            yps[:, :il],
                            lhsT=wo_sb[:, h, oc * 128 : (oc + 1) * 128],
                            rhs=outT[h][:, ns : ns + il],
                            start=(h == 0),
                            stop=(h == 3),
                        )
                    y_sb = mp.tile([128, 512], f32, tag="ysb", name="y_sb")
                    nc.vector.tensor_copy(y_sb[:, :il], yps[:, :il])
                    nc.sync.dma_start(
                        out=y_d[oc * 128 : (oc + 1) * 128, ns : ns + il],
                        in_=y_sb[:, :il],
                    )

    return nc


_NC_CACHE = None


def kernel(x, w_qkv, w_out, b_out):
    global _NC_CACHE
    from concourse.bass_utils import run_bass_kernel_spmd

    x = np.ascontiguousarray(x, dtype=np.float32)
    w_qkv = np.asarray(w_qkv, dtype=np.float32)
    w_out = np.asarray(w_out, dtype=np.float32)
    b_out = np.asarray(b_out, dtype=np.float32)

    b, c, h, w = x.shape
    assert (b, c, h, w) == (B, C, 48, 48)
    x_bn = x.reshape(B, C, N)

    wq, wk, wv = w_qkv[0:HID], w_qkv[HID : 2 * HID], w_qkv[2 * HID : 3 * HID]
    w_outT = np.ascontiguousarray(w_out.T)  # [HID, C]

    in_maps = []
    for core in range(N_CORES):
        bb, g = core // 2, core % 2
        rows = slice(g * 256, g * 256 + 256)
        woutT_c = np.ascontiguousarray(
            w_outT[rows].reshape(4, 64, 256).transpose(1, 0, 2)
        )
        in_maps.append(
            {
                "x": np.ascontiguousarray(x_bn[bb]),
                "wqT": np.ascontiguousarray(wq[rows].T),
                "wkT": np.ascontiguousarray(wk[rows].T),
                "wvT": np.ascontiguousarray(wv[rows].T),
                "woutT": woutT_c,
            }
        )

    if _NC_CACHE is None:
        _NC_CACHE = build_kernel()
    nc = _NC_CACHE

    trace = bool(int(os.environ.get("KERNEL_TRACE", "0")))
    res = run_bass_kernel_spmd(
        nc,
        in_maps,
        core_ids=list(range(N_CORES)),
        trace=trace,
        trace_cores=list(range(N_CORES)) if trace else None,
    )
    kernel.last_result = res

    y = np.empty((B, C, N), dtype=np.float32)
    for bb in range(B):
        y[bb] = (
            res.results[2 * bb]["y"]
            + res.results[2 * bb + 1]["y"]
            + b_out[:, None]
        )
    return y.reshape(B, C, 48, 48)


# revision 11
# speedup vs baseline: 1.1054x; 1.1054x over previous
"""Trainium2 Bass kernel for nn_Attention_16028817948779.

Reference computation (b=4, c=256, heads=8, d=64, h=w=48, n=2304):
  qkv = w_qkv @ x          (1x1 conv)
  q,k,v -> [b, H, d, n];  q,k l2-normalized along n (spatial)
  sim  = (q^T k) * 10;  attn = softmax(sim, axis=-1)
  out  = attn @ v^T -> [b, H, n, d] -> [b, H*d, h, w]
  y    = w_out @ out + b_out

Sharding: 8 cores; core c handles batch c//2, head group (c%2)*4..+4.
Each core computes a partial y over its 4 heads; host sums the two
partials per batch and adds the bias.

Kernel design:
  - All big matmuls in float32r (1 PE cycle/row when free dim >= 256).
  - Attention in transposed form ST[j,i] = k_j . q_i, so softmax needs no
    on-chip transposes: exp without max-subtraction is safe because q,k
    are l2-normalized along n (|sim| <~ 0.7) and the softmax denominator
    comes for free from a ones-column appended to V^T in the PV matmul.
  - Two heads' ST matmuls run concurrently on the PE via row tiling
    (K=64 each; the head pair lives at partitions 0-63 / 64-127 of the
    QKV projection output).
  - 10/(|q_row| |k_row|) folded into a single per-row scale of q.
"""

import os
import sys

import numpy as np

_TRN_REPO = "/opt/trn_rl_repo"
if _TRN_REPO not in sys.path:
    sys.path.insert(0, _TRN_REPO)

B = 4
C = 256
HEADS = 8
D = 64
N = 2304  # 48*48
HID = HEADS * D  # 512

N_CORES = 8
CI = 2  # c chunks of 128
# i/n chunks of <=512 (PSUM bank / fp32 moving-operand limit)
NCHUNKS = [(0, 512), (512, 512), (1024, 512), (1536, 512), (2048, 256)]
NJ = N // 128  # 18 key chunks of 128


def _apply_compat_patches():
    """walrus in this env only accepts ~1 sync wait per instruction, but the
    Tile framework attaches one wait per outstanding proc to a single
    instruction. Split excess waits onto EventSemaphore instructions at the
    BIR-JSON level (Bass.to_json_bytes is the serialization choke point for
    both the native and the axon/PJRT compile paths)."""
    import json

    import concourse.bass as bass

    if getattr(bass.Bass.to_json_bytes, "_waitsplit", False):
        return

    MAXW = 1
    _orig = bass.Bass.to_json_bytes

    def _split_waits(raw):
        m = json.loads(raw)
        ctr = 0
        changed = False
        for f in m.get("functions", []):
            for blk in f.get("blocks", []):
                new_insts = []
                for ins in blk.get("instructions", []):
                    si = ins.get("sync_info")
                    waits = (si or {}).get("on_wait") or []
                    if len(waits) > MAXW:
                        changed = True
                        for w in waits[:-MAXW]:
                            ctr += 1
                            new_insts.append(
                                {
                                    "debug": ins.get("debug", 0),
                                    "engine": ins["engine"],
                                    "ins": [],
                                    "outs": [],
                                    "name": f"waitsplit_{ctr}",
                                    "opcode": "EventSemaphore",
                                    "sync_info": {"on_update": [], "on_wait": [w]},
                                }
                            )
                        si["on_wait"] = waits[-MAXW:]
                    new_insts.append(ins)
                blk["instructions"] = new_insts
        return json.dumps(m).encode() if changed else raw

    def _patched(self):
        return _split_waits(_orig(self))

    _patched._waitsplit = True
    bass.Bass.to_json_bytes = _patched


def build_kernel():
    import concourse.bass as bass
    import concourse.mybir as mybir
    import concourse.tile as tile

    _apply_compat_patches()

    f32 = mybir.dt.float32
    f32r = mybir.dt.float32r
    Exp = mybir.ActivationFunctionType.Exp
    Sqrt = mybir.ActivationFunctionType.Sqrt
    Square = mybir.ActivationFunctionType.Square
    mult = mybir.AluOpType.mult
    X = mybir.AxisListType.X

    nc = bass.Bass()
    x_d = nc.dram_tensor("x", [C, N], f32r, kind="ExternalInput")
    wqT_d = nc.dram_tensor("wqT", [C, 256], f32r, kind="ExternalInput")
    wkT_d = nc.dram_tensor("wkT", [C, 256], f32r, kind="ExternalInput")
    wvT_d = nc.dram_tensor("wvT", [C, 256], f32r, kind="ExternalInput")
    woutT_d = nc.dram_tensor("woutT", [64, 4, 256], f32r, kind="ExternalInput")
    y_d = nc.dram_tensor("y", [C, N], f32, kind="ExternalOutput")

    with tile.TileContext(nc) as tc:
        with (
            tc.tile_pool(name="persist", bufs=1) as pp,
            tc.tile_pool(name="pt", bufs=3) as ptp,
            tc.tile_pool(name="misc", bufs=2) as mp,
            tc.tile_pool(name="dram", bufs=2, space="DRAM") as dp,
            tc.tile_pool(name="ps_st", bufs=2, space="PSUM") as ps_st,
            tc.tile_pool(name="ps_pv", bufs=2, space="PSUM") as ps_pv,
        ):
            # ---- load inputs ----
            x_sb = pp.tile([128, CI, N], f32r)
            for ci in range(CI):
                nc.sync.dma_start(
                    out=x_sb[:, ci, :], in_=x_d[ci * 128 : (ci + 1) * 128, :]
                )
            wq_sb = pp.tile([128, CI, 256], f32r)
            wk_sb = pp.tile([128, CI, 256], f32r)
            wv_sb = pp.tile([128, CI, 256], f32r)
            for w_sb, w_d in ((wq_sb, wqT_d), (wk_sb, wkT_d), (wv_sb, wvT_d)):
                nc.sync.dma_start(
                    out=w_sb[:], in_=w_d.rearrange("(ci p) o -> p ci o", p=128)
                )
            wo_sb = pp.tile([64, 4, 256], f32r)
            nc.sync.dma_start(out=wo_sb[:], in_=woutT_d[:])

            ones_f = pp.tile([128, 1], f32)
            nc.vector.memset(ones_f[:], 1.0)

            # ---- QKV projection ----
            # q_sb/k_sb: [d-part, head-pair, n]; heads 2p at part 0-63, 2p+1 at 64-127
            q_sb = pp.tile([128, 2, N], f32r)
            k_sb = pp.tile([128, 2, N], f32r)
            # per-(tensor, oc) partial sum-of-squares, one column per n-chunk,
            # computed chunk-wise right behind the projection so the PE never
            # waits on the l2norm reduction later
            ssq = mp.tile([128, 2, 2, len(NCHUNKS)], f32, tag="ssq")
            scratch = pp.tile([128, 512], f32)
            for ti, (dst, w_sb) in enumerate(((q_sb, wq_sb), (k_sb, wk_sb))):
                for oc in range(2):
                    for nci, (ns, nl) in enumerate(NCHUNKS):
                        ps = ps_pv.tile([128, 512], f32, tag="pvA", name="qk_ps")
                        for ci in range(CI):
                            nc.tensor.matmul(
                                ps[:, :nl],
                                lhsT=w_sb[:, ci, oc * 128 : (oc + 1) * 128],
                                rhs=x_sb[:, ci, ns : ns + nl],
                                start=(ci == 0),
                                stop=(ci == CI - 1),
                            )
                        nc.vector.tensor_copy(dst[:, oc, ns : ns + nl], ps[:, :nl])
                        nc.scalar.activation(
                            scratch[:, :nl],
                            ps[:, :nl],
                            Square,
                            accum_out=ssq[:, ti, oc, nci : nci + 1],
                        )

            # ---- fold l2norm + SCALE into q: q *= 10/sqrt(ssq_q*ssq_k) per row ----
            sqk = mp.tile([128, 2, 2], f32, tag="sqk")
            nc.vector.reduce_sum(
                sqk.rearrange("p a b -> p (a b)"),
                ssq.rearrange("p a b c -> p (a b) c"),
                axis=X,
            )
            qscale = mp.tile([128, 2], f32, tag="qscale")
            nc.vector.tensor_tensor(qscale[:], sqk[:, 0, :], sqk[:, 1, :], mult)
            # sqrt(0.01*sq*sk) = 0.1*sqrt(..) ; recip -> 10/sqrt(..)
            nc.scalar.activation(qscale[:], qscale[:], Sqrt, scale=0.01)
            nc.vector.reciprocal(qscale[:], qscale[:])
            with nc.allow_low_precision(reason="q scale written as fp32r"):
                for oc in range(2):
                    nc.vector.tensor_scalar_mul(
                        q_sb[:, oc, :], q_sb[:, oc, :], qscale[:, oc : oc + 1]
                    )

            # vt_sb: [n-part, j-chunk, 4*65]; per head 64 v columns + a ones col
            vt_sb = pp.tile([128, NJ, 260], f32r)
            vt4 = vt_sb.rearrange("p j (h e) -> p j h e", e=65)
            nc.vector.tensor_copy(
                vt4[:, :, :, 64:65],
                ones_f[:, 0:1].unsqueeze(1).unsqueeze(1).to_broadcast((128, NJ, 4, 1)),
            )
            for j in range(NJ):
                ps = ps_pv.tile([128, 256], f32, tag="pvB", name="vt_ps")
                for ci in range(CI):
                    nc.tensor.matmul(
                        ps[:],
                        lhsT=x_sb[:, ci, j * 128 : (j + 1) * 128],
                        rhs=wv_sb[:, ci, :],
                        start=(ci == 0),
                        stop=(ci == CI - 1),
                    )
                nc.vector.tensor_copy(
                    vt4[:, j, :, 0:64], ps.rearrange("p (h d) -> p h d", h=4)
                )

            # ---- attention per head pair p (local heads 2p, 2p+1) ----
            outT = [
                pp.tile([64, N], f32r, name=f"outT{h}", tag=f"outT{h}")
                for h in range(4)
            ]
            for p in range(2):
                for nci, (ns, il) in enumerate(NCHUNKS):
                    hA, hB = 2 * p, 2 * p + 1
                    pvA = ps_pv.tile([65, 512], f32, tag="pvA", name="pvA")
                    pvB = ps_pv.tile([65, 512], f32, tag="pvB", name="pvB")
                    for j in range(NJ):
                        st = ps_st.tile([128, 2, 512], f32, tag="st", name="st")
                        nc.tensor.matmul(
                            st[:, 0, :il],
                            lhsT=k_sb[0:64, p, j * 128 : (j + 1) * 128],
                            rhs=q_sb[0:64, p, ns : ns + il],
                        )
                        nc.tensor.matmul(
                            st[:, 1, :il],
                            lhsT=k_sb[64:128, p, j * 128 : (j + 1) * 128],
                            rhs=q_sb[64:128, p, ns : ns + il],
                        )
                        pt = ptp.tile([128, 2, 512], f32r, tag="pt", name="pt")
                        nc.scalar.activation(pt[:, :, :il], st[:, :, :il], Exp)
                        nc.tensor.matmul(
                            pvA[:, :il],
                            lhsT=vt4[:, j, hA, :],
                            rhs=pt[:, 0, :il],
                            start=(j == 0),
                            stop=(j == NJ - 1),
                        )
                        nc.tensor.matmul(
                            pvB[:, :il],
                            lhsT=vt4[:, j, hB, :],
                            rhs=pt[:, 1, :il],
                            start=(j == 0),
                            stop=(j == NJ - 1),
                        )
                    # Copy each finished PV accumulator to SBUF right away so
                    # the PSUM slot frees for the next i-block, then normalize
                    # rows 0-63 by row 64 (softmax denominator) from SBUF; the
                    # [1,il] reciprocal row is partition-replicated via DMA.
                    for pv, h in ((pvA, hA), (pvB, hB)):
                        nsb = mp.tile([65, 512], f32, tag="nsb", name="nsb")
                        nc.vector.tensor_copy(nsb[:, :il], pv[:, :il])
                        rden_f = mp.tile([1, 512], f32, tag="rdenf", name="rden_f")
                        nc.vector.reciprocal(rden_f[:, :il], nsb[64:65, :il])
                        rden_d = dp.tile([1, 512], f32, tag="rdend", name="rden_d")
                        nc.sync.dma_start(out=rden_d[:, :il], in_=rden_f[:, :il])
                        bc_sb = mp.tile([64, 512], f32, tag="bcsb", name="bc_sb")
                        nc.sync.dma_start(
                            out=bc_sb[:, :il],
                            in_=rden_d[0:1, :il].to_broadcast((64, il)),
                        )
                        nc.vector.tensor_tensor(
                            outT[h][:, ns : ns + il],
                            nsb[0:64, :il],
                            bc_sb[:, :il],
                            mult,
                        )

                    # ---- output projection, interleaved into the p=1 stream;
                    # the final i-block's projection lands after the loop ----
                    if p == 1:
                        for oc in range(2):
                            yps = ps_pv.tile([128, 512], f32, tag="pvA", name="yps")
                            for h in range(4):
                                nc.tensor.matmul(
                                    yps[:, :il],
                                    lhsT=wo_sb[:, h, oc * 128 : (oc + 1) * 128],
                                    rhs=outT[h][:, ns : ns + il],
                                    start=(h == 0),
                                    stop=(h == 3),
                                )
                            y_sb = mp.tile([128, 512], f32, tag="ysb", name="y_sb")
                            nc.vector.tensor_copy(y_sb[:, :il], yps[:, :il])
                            nc.sync.dma_start(
                                out=y_d[oc * 128 : (oc + 1) * 128, ns : ns + il],
                                in_=y_sb[:, :il],
                            )

    return nc


_NC_CACHE = None


def kernel(x, w_qkv, w_out, b_out):
    global _NC_CACHE
    from concourse.bass_utils import run_bass_kernel_spmd

    x = np.ascontiguousarray(x, dtype=np.float32)
    w_qkv = np.asarray(w_qkv, dtype=np.float32)
    w_out = np.asarray(w_out, dtype=np.float32)
    b_out = np.asarray(b_out, dtype=np.float32)

    b, c, h, w = x.shape
    assert (b, c, h, w) == (B, C, 48, 48)
    x_bn = x.reshape(B, C, N)

    wq, wk, wv = w_qkv[0:HID], w_qkv[HID : 2 * HID], w_qkv[2 * HID : 3 * HID]
    w_outT = np.ascontiguousarray(w_out.T)  # [HID, C]

    in_maps = []
    for core in range(N_CORES):
        bb, g = core // 2, core % 2
        rows = slice(g * 256, g * 256 + 256)
        woutT_c = np.ascontiguousarray(
            w_outT[rows].reshape(4, 64, 256).transpose(1, 0, 2)
        )
        in_maps.append(
            {
                "x": np.ascontiguousarray(x_bn[bb]),
                "wqT": np.ascontiguousarray(wq[rows].T),
                "wkT": np.ascontiguousarray(wk[rows].T),
                "wvT": np.ascontiguousarray(wv[rows].T),
                "woutT": woutT_c,
            }
        )

    if _NC_CACHE is None:
        _NC_CACHE = build_kernel()
    nc = _NC_CACHE

    trace = bool(int(os.environ.get("KERNEL_TRACE", "0")))
    res = run_bass_kernel_spmd(
        nc,
        in_maps,
        core_ids=list(range(N_CORES)),
        trace=trace,
        trace_cores=list(range(N_CORES)) if trace else None,
    )
    kernel.last_result = res

    y = np.empty((B, C, N), dtype=np.float32)
    for bb in range(B):
        y[bb] = (
            res.results[2 * bb]["y"]
            + res.results[2 * bb + 1]["y"]
            + b_out[:, None]
        )
    return y.reshape(B, C, 48, 48)


# revision 16
# speedup vs baseline: 1.5080x; 1.3642x over previous
"""Trainium2 Bass kernel for nn_Attention_16028817948779.

Reference computation (b=4, c=256, heads=8, d=64, h=w=48, n=2304):
  qkv = w_qkv @ x          (1x1 conv)
  q,k,v -> [b, H, d, n];  q,k l2-normalized along n (spatial)
  sim  = (q^T k) * 10;  attn = softmax(sim, axis=-1)
  out  = attn @ v^T -> [b, H, n, d] -> [b, H*d, h, w]
  y    = w_out @ out + b_out

Sharding: 8 cores; core c handles batch c//2, head group (c%2)*4..+4.
Each core computes a partial y over its 4 heads; host sums the two
partials per batch and adds the bias.

Kernel design:
  - All big matmuls in float32r (1 PE cycle/row when free dim >= 256).
  - Attention in transposed form ST[j,i] = k_j . q_i, so softmax needs no
    on-chip transposes: exp without max-subtraction is safe because q,k
    are l2-normalized along n (|sim| <~ 0.7) and the softmax denominator
    comes for free from a ones-column appended to V^T in the PV matmul.
  - Two heads' ST matmuls run concurrently on the PE via row tiling
    (K=64 each; the head pair lives at partitions 0-63 / 64-127 of the
    QKV projection output).
  - 10/(|q_row| |k_row|) folded into a single per-row scale of q.
"""

import os
import sys

import numpy as np

_TRN_REPO = "/opt/trn_rl_repo"
if _TRN_REPO not in sys.path:
    sys.path.insert(0, _TRN_REPO)

B = 4
C = 256
HEADS = 8
D = 64
N = 2304  # 48*48
HID = HEADS * D  # 512

N_CORES = 8
CI = 2  # c chunks of 128
# i/n chunks of <=512 (PSUM bank / fp32 moving-operand limit)
NCHUNKS = [(0, 512), (512, 512), (1024, 512), (1536, 512), (2048, 256)]
NJ = N // 128  # 18 key chunks of 128


def _apply_compat_patches():
    """walrus in this env only accepts ~1 sync wait per instruction, but the
    Tile framework attaches one wait per outstanding proc to a single
    instruction. Split excess waits onto EventSemaphore instructions at the
    BIR-JSON level (Bass.to_json_bytes is the serialization choke point for
    both the native and the axon/PJRT compile paths)."""
    import json

    import concourse.bass as bass

    if getattr(bass.Bass.to_json_bytes, "_waitsplit", False):
        return

    MAXW = 1
    _orig = bass.Bass.to_json_bytes

    def _split_waits(raw):
        m = json.loads(raw)
        ctr = 0
        changed = False
        for f in m.get("functions", []):
            for blk in f.get("blocks", []):
                new_insts = []
                for ins in blk.get("instructions", []):
                    si = ins.get("sync_info")
                    waits = (si or {}).get("on_wait") or []
                    if len(waits) > MAXW:
                        changed = True
                        for w in waits[:-MAXW]:
                            ctr += 1
                            new_insts.append(
                                {
                                    "debug": ins.get("debug", 0),
                                    "engine": ins["engine"],
                                    "ins": [],
                                    "outs": [],
                                    "name": f"waitsplit_{ctr}",
                                    "opcode": "EventSemaphore",
                                    "sync_info": {"on_update": [], "on_wait": [w]},
                                }
                            )
                        si["on_wait"] = waits[-MAXW:]
                    new_insts.append(ins)
                blk["instructions"] = new_insts
        return json.dumps(m).encode() if changed else raw

    def _patched(self):
        return _split_waits(_orig(self))

    _patched._waitsplit = True
    bass.Bass.to_json_bytes = _patched


def build_kernel():
    import concourse.bass as bass
    import concourse.mybir as mybir
    import concourse.tile as tile

    _apply_compat_patches()

    f32 = mybir.dt.float32
    f32r = mybir.dt.float32r
    Exp = mybir.ActivationFunctionType.Exp
    Sqrt = mybir.ActivationFunctionType.Sqrt
    Square = mybir.ActivationFunctionType.Square
    mult = mybir.AluOpType.mult
    X = mybir.AxisListType.X

    nc = bass.Bass()
    x_d = nc.dram_tensor("x", [C, N], f32r, kind="ExternalInput")
    wqT_d = nc.dram_tensor("wqT", [C, 256], f32r, kind="ExternalInput")
    wkT_d = nc.dram_tensor("wkT", [C, 256], f32r, kind="ExternalInput")
    wvT_d = nc.dram_tensor("wvT", [C, 256], f32r, kind="ExternalInput")
    woutT_d = nc.dram_tensor("woutT", [64, 4, 256], f32r, kind="ExternalInput")
    y_d = nc.dram_tensor("y", [C, N], f32, kind="ExternalOutput")

    with tile.TileContext(nc) as tc:
        with (
            tc.tile_pool(name="persist", bufs=1) as pp,
            tc.tile_pool(name="pt", bufs=3) as ptp,
            tc.tile_pool(name="misc", bufs=2) as mp,
            tc.tile_pool(name="dram", bufs=2, space="DRAM") as dp,
            tc.tile_pool(name="ps_st", bufs=3, space="PSUM") as ps_st,
            tc.tile_pool(name="ps_pv", bufs=2, space="PSUM") as ps_pv,
        ):
            # ---- load inputs ----
            x_sb = pp.tile([128, CI, N], f32r)
            for ci in range(CI):
                for ns, nl in NCHUNKS:
                    nc.sync.dma_start(
                        out=x_sb[:, ci, ns : ns + nl],
                        in_=x_d[ci * 128 : (ci + 1) * 128, ns : ns + nl],
                    )
            wq_sb = pp.tile([128, CI, 256], f32r)
            wk_sb = pp.tile([128, CI, 256], f32r)
            wv_sb = pp.tile([128, CI, 256], f32r)
            for w_sb, w_d in ((wq_sb, wqT_d), (wk_sb, wkT_d), (wv_sb, wvT_d)):
                nc.sync.dma_start(
                    out=w_sb[:], in_=w_d.rearrange("(ci p) o -> p ci o", p=128)
                )
            wo_sb = pp.tile([64, 4, 256], f32r)
            nc.sync.dma_start(out=wo_sb[:], in_=woutT_d[:])

            ones_f = pp.tile([128, 1], f32)
            nc.vector.memset(ones_f[:], 1.0)

            # ---- QKV projection ----
            # q_sb/k_sb: [d-part, head-pair, n]; heads 2p at part 0-63, 2p+1 at 64-127
            q_sb = pp.tile([128, 2, N], f32r)
            k_sb = pp.tile([128, 2, N], f32r)
            # per-(tensor, oc) partial sum-of-squares, one column per n-chunk,
            # computed chunk-wise right behind the projection so the PE never
            # waits on the l2norm reduction later
            ssq = mp.tile([128, 2, 2, len(NCHUNKS)], f32, tag="ssq")
            scratch = pp.tile([128, 512], f32)
            for ti, (dst, w_sb) in enumerate(((q_sb, wq_sb), (k_sb, wk_sb))):
                for oc in range(2):
                    for nci, (ns, nl) in enumerate(NCHUNKS):
                        ps = ps_pv.tile([128, 512], f32, tag="pv", name="qk_ps")
                        for ci in range(CI):
                            nc.tensor.matmul(
                                ps[:, :nl],
                                lhsT=w_sb[:, ci, oc * 128 : (oc + 1) * 128],
                                rhs=x_sb[:, ci, ns : ns + nl],
                                start=(ci == 0),
                                stop=(ci == CI - 1),
                            )
                        nc.vector.tensor_copy(dst[:, oc, ns : ns + nl], ps[:, :nl])
                        nc.scalar.activation(
                            scratch[:, :nl],
                            ps[:, :nl],
                            Square,
                            accum_out=ssq[:, ti, oc, nci : nci + 1],
                        )

            # ---- fold l2norm + SCALE into q: q *= 10/sqrt(ssq_q*ssq_k) per row ----
            sqk = mp.tile([128, 2, 2], f32, tag="sqk")
            nc.vector.reduce_sum(
                sqk.rearrange("p a b -> p (a b)"),
                ssq.rearrange("p a b c -> p (a b) c"),
                axis=X,
            )
            qscale = mp.tile([128, 2], f32, tag="qscale")
            nc.vector.tensor_tensor(qscale[:], sqk[:, 0, :], sqk[:, 1, :], mult)
            # sqrt(0.01*sq*sk) = 0.1*sqrt(..) ; recip -> 10/sqrt(..)
            nc.scalar.activation(qscale[:], qscale[:], Sqrt, scale=0.01)
            nc.vector.reciprocal(qscale[:], qscale[:])
            with nc.allow_low_precision(reason="q scale written as fp32r"):
                # chunk-split so the first ST matmuls only wait on chunk 0
                for ns, nl in NCHUNKS:
                    for oc in range(2):
                        nc.vector.tensor_scalar_mul(
                            q_sb[:, oc, ns : ns + nl],
                            q_sb[:, oc, ns : ns + nl],
                            qscale[:, oc : oc + 1],
                        )

            # vt_sb: [n-part, j-chunk, 4*65]; per head 64 v columns + a ones col
            vt_sb = pp.tile([128, NJ, 260], f32r)
            vt4 = vt_sb.rearrange("p j (h e) -> p j h e", e=65)
            nc.vector.tensor_copy(
                vt4[:, :, :, 64:65],
                ones_f[:, 0:1].unsqueeze(1).unsqueeze(1).to_broadcast((128, NJ, 4, 1)),
            )
            for j in range(NJ):
                ps = ps_pv.tile([128, 256], f32, tag="pv", name="vt_ps")
                for ci in range(CI):
                    nc.tensor.matmul(
                        ps[:],
                        lhsT=x_sb[:, ci, j * 128 : (j + 1) * 128],
                        rhs=wv_sb[:, ci, :],
                        start=(ci == 0),
                        stop=(ci == CI - 1),
                    )
                nc.vector.tensor_copy(
                    vt4[:, j, :, 0:64], ps.rearrange("p (h d) -> p h d", h=4)
                )

            # ---- attention per head pair p (local heads 2p, 2p+1) ----
            outT = [
                pp.tile([64, N], f32r, name=f"outT{h}", tag=f"outT{h}")
                for h in range(4)
            ]
            def emit_proj(ns, il):
                for oc in range(2):
                    yps = ps_pv.tile([128, 512], f32, tag="pv", name="yps")
                    for h in range(4):
                        nc.tensor.matmul(
                            yps[:, :il],
                            lhsT=wo_sb[:, h, oc * 128 : (oc + 1) * 128],
                            rhs=outT[h][:, ns : ns + il],
                            start=(h == 0),
                            stop=(h == 3),
                        )
                    y_sb = mp.tile([128, 512], f32, tag="ysb", name="y_sb")
                    nc.vector.tensor_copy(y_sb[:, :il], yps[:, :il])
                    nc.sync.dma_start(
                        out=y_d[oc * 128 : (oc + 1) * 128, ns : ns + il],
                        in_=y_sb[:, :il],
                    )

            for p in range(2):
                for nci, (ns, il) in enumerate(NCHUNKS):
                    # During p=1, project the PREVIOUS i-block: its normalize
                    # chain has had a full block of slack, so these matmuls
                    # never stall the PE.
                    if p == 1 and nci > 0:
                        emit_proj(*NCHUNKS[nci - 1])
                    hA, hB = 2 * p, 2 * p + 1
                    pvA = ps_pv.tile([65, 512], f32, tag="pv", name="pvA")
                    pvB = ps_pv.tile([65, 512], f32, tag="pv", name="pvB")
                    for j in range(NJ):
                        st = ps_st.tile([128, 2, 512], f32, tag="st", name="st")
                        nc.tensor.matmul(
                            st[:, 0, :il],
                            lhsT=k_sb[0:64, p, j * 128 : (j + 1) * 128],
                            rhs=q_sb[0:64, p, ns : ns + il],
                        )
                        nc.tensor.matmul(
                            st[:, 1, :il],
                            lhsT=k_sb[64:128, p, j * 128 : (j + 1) * 128],
                            rhs=q_sb[64:128, p, ns : ns + il],
                        )
                        pt = ptp.tile([128, 2, 512], f32r, tag="pt", name="pt")
                        nc.scalar.activation(pt[:, :, :il], st[:, :, :il], Exp)
                        nc.tensor.matmul(
                            pvA[:, :il],
                            lhsT=vt4[:, j, hA, :],
                            rhs=pt[:, 0, :il],
                            start=(j == 0),
                            stop=(j == NJ - 1),
                        )
                        nc.tensor.matmul(
                            pvB[:, :il],
                            lhsT=vt4[:, j, hB, :],
                            rhs=pt[:, 1, :il],
                            start=(j == 0),
                            stop=(j == NJ - 1),
                        )
                    # Copy each finished PV accumulator to SBUF right away so
                    # the PSUM slot frees for the next i-block, then normalize
                    # rows 0-63 by row 64 (softmax denominator) from SBUF; the
                    # [1,il] reciprocal row is partition-replicated via DMA.
                    for pv, h in ((pvA, hA), (pvB, hB)):
                        nsb = mp.tile([65, 512], f32, tag="nsb", name="nsb")
                        nc.vector.tensor_copy(nsb[:, :il], pv[:, :il])
                        rden_f = mp.tile([1, 512], f32, tag="rdenf", name="rden_f")
                        nc.vector.reciprocal(rden_f[:, :il], nsb[64:65, :il])
                        rden_d = dp.tile([1, 512], f32, tag="rdend", name="rden_d")
                        nc.sync.dma_start(out=rden_d[:, :il], in_=rden_f[:, :il])
                        bc_sb = mp.tile([64, 512], f32, tag="bcsb", name="bc_sb")
                        nc.sync.dma_start(
                            out=bc_sb[:, :il],
                            in_=rden_d[0:1, :il].to_broadcast((64, il)),
                        )
                        nc.vector.tensor_tensor(
                            outT[h][:, ns : ns + il],
                            nsb[0:64, :il],
                            bc_sb[:, :il],
                            mult,
                        )


            emit_proj(*NCHUNKS[-1])

    return nc


_NC_CACHE = None


def kernel(x, w_qkv, w_out, b_out):
    global _NC_CACHE
    from concourse.bass_utils import run_bass_kernel_spmd

    x = np.ascontiguousarray(x, dtype=np.float32)
    w_qkv = np.asarray(w_qkv, dtype=np.float32)
    w_out = np.asarray(w_out, dtype=np.float32)
    b_out = np.asarray(b_out, dtype=np.float32)

    b, c, h, w = x.shape
    assert (b, c, h, w) == (B, C, 48, 48)
    x_bn = x.reshape(B, C, N)

    wq, wk, wv = w_qkv[0:HID], w_qkv[HID : 2 * HID], w_qkv[2 * HID : 3 * HID]
    w_outT = np.ascontiguousarray(w_out.T)  # [HID, C]

    in_maps = []
    for core in range(N_CORES):
        bb, g = core // 2, core % 2
        rows = slice(g * 256, g * 256 + 256)
        woutT_c = np.ascontiguousarray(
            w_outT[rows].reshape(4, 64, 256).transpose(1, 0, 2)
        )
        in_maps.append(
            {
                "x": np.ascontiguousarray(x_bn[bb]),
                "wqT": np.ascontiguousarray(wq[rows].T),
                "wkT": np.ascontiguousarray(wk[rows].T),
                "wvT": np.ascontiguousarray(wv[rows].T),
                "woutT": woutT_c,
            }
        )

    if _NC_CACHE is None:
        _NC_CACHE = build_kernel()
    nc = _NC_CACHE

    trace = bool(int(os.environ.get("KERNEL_TRACE", "0")))
    res = run_bass_kernel_spmd(
        nc,
        in_maps,
        core_ids=list(range(N_CORES)),
        trace=trace,
        trace_cores=list(range(N_CORES)) if trace else None,
    )
    kernel.last_result = res

    y = np.empty((B, C, N), dtype=np.float32)
    for bb in range(B):
        y[bb] = (
            res.results[2 * bb]["y"]
            + res.results[2 * bb + 1]["y"]
            + b_out[:, None]
        )
    return y.reshape(B, C, 48, 48)
